# revision 6
# baseline (speedup 1.0000x reference)
"""GAT layer (nn_GATLayer) on 8 Trainium2 NeuronCores via Bass/Tile.

Reference computation (N=8192, F=512, D=64):
    z = features @ W                      # [N, D]
    s = z @ a_self; t = z @ a_neigh       # [N, 1]
    e[i,j] = leakyrelu(s[i] + t[j], 0.2)
    attention = softmax(e + mask(A), axis=1)   # mask: -1e12 where A<=0
    h = attention @ z                     # [N, D]

Row-sharded across 8 cores (1024 attention rows each), two launches with a
host-side gather between them (replaces an on-device AllGather whose
rendezvous barrier alone costs ~47 us).

Key algebra: with e = s_i + t_j, q_j = exp(.8 t_j), p_i = exp(.8 s_i),
    exp(leakyrelu(e)) = exp(.2 t_j) * exp(.2 s_i) * max(p_i q_j, 1).
The exp(.2 s_i) column factor cancels between softmax numerator and
denominator and is dropped; exp(.2 t_j) is folded into the stationary
z' = [z | 1] * exp(.2 t_j) in launch A.  Per-tile mask weight is then
    ea = A * max(pq, 1)
and the crucial split (scheme Y/Z below)
    ea = A + relu(pq - 1) * A
lets the raw A tile stream straight from DMA into the PE (zero elementwise
cost) with the relu term as a second moving operand into the SAME psum
accumulator + stationary.  relu(pq - 1) is ONE scalar-engine activation:
Relu(p3 * eq_j - 1) via the per-partition `scale` operand.

Launch A (small): each core computes z (bf16 one-pass + W-lo correction),
t, s for its own 1024 rows; ships z' pre-scaled/cast to f16, eq = exp(.8t)
f16, s row f32.

Launch B (main): per j-chunk (128 j's x 1024 i's), one of three schemes
balances the elementwise mask work across engines under the A-DMA roofline:
  X: m  = max(p3*eq_j, 1)        (DVE tensor_scalar, 1 op)
     ea = m * A                  (DVE tensor_tensor) -> 1 moving stream
  Y: r  = Relu(p3*eq_j - 1)      (ACT activation, 1 op)
     g  = r * A                  (DVE tensor_tensor) -> 2 moving streams
  W/Z: same as X/Y but the tensor_tensor runs on GpSimd.
A-tiles arrive as 1 MB DMAs (4 chunks each) alternating sync/scalar HWDGE
queues; A ships as float16 0/1 (exact), host-prepacked to the on-chip
layout [128, JC*1024].
"""

import sys

sys.path.insert(0, "/opt/trn_rl_repo")

import numpy as np

N, F, D = 8192, 512, 64
NCORES = 8
R = N // NCORES          # rows per core (1024)
JC = N // 128            # j-chunks (64)
DP = D + 1               # z' | et2  (65)
ZW = 80                  # padded z' width (80 f16 = 160B rows)
ALPHA = 0.2
CPD = 4                  # chunks per A-DMA (4 * 256KB = 1MB)

# per-16-chunk scheme pattern:
#   'X' = DVE ts + DVE tt (1 stream)   'W' = DVE ts + GpS tt (1 stream)
#   'Y' = ACT relu + DVE tt (2 streams) 'Z' = ACT relu + GpS tt (2 streams)
SCHEME16 = ['X', 'Y', 'Z', 'Y', 'X', 'Y', 'W', 'X',
            'Y', 'Y', 'X', 'Y', 'Z', 'X', 'Y', 'Y']

_CACHE = {}


def _build_launch_a():
    """Per-core z' = [z|1]*exp(.2 t) f16, eq = exp(.8 t) f16, s row f32."""
    import concourse.bacc as bacc
    import concourse.tile as tile
    from concourse import mybir
    from concourse.masks import make_identity

    f32 = mybir.dt.float32
    f16 = mybir.dt.float16
    bf16 = mybir.dt.bfloat16
    Alu = mybir.AluOpType
    Act = mybir.ActivationFunctionType

    nc = bacc.Bacc("TRN2", target_bir_lowering=False, debug=False, num_devices=NCORES)

    feat_t = nc.dram_tensor("feat_t", [F, R], bf16, kind="ExternalInput")
    w_in = nc.dram_tensor("w", [F, D], f32, kind="ExternalInput")
    a_self = nc.dram_tensor("a_self", [1, D], f32, kind="ExternalInput")
    a_neigh = nc.dram_tensor("a_neigh", [1, D], f32, kind="ExternalInput")
    za_out = nc.dram_tensor("za", [R, ZW], f16, kind="ExternalOutput")
    s_out = nc.dram_tensor("s", [1, R], f32, kind="ExternalOutput")
    eq_out = nc.dram_tensor("eq", [128, R // 128], f32, kind="ExternalOutput")

    IB = R // 128  # 8 row-blocks per core

    with tile.TileContext(nc) as tc:
        with (
            tc.tile_pool(name="sb", bufs=1) as cst,
            tc.tile_pool(name="ps", bufs=2, space="PSUM") as ps,
        ):
            ft = cst.tile([128, 4 * R], bf16)
            for c in range(4):
                nc.sync.dma_start(out=ft[:, c * R:(c + 1) * R],
                                  in_=feat_t[c * 128:(c + 1) * 128, :])
            w_sb = cst.tile([128, 4 * D], f32)
            for c in range(4):
                nc.scalar.dma_start(out=w_sb[:, c * D:(c + 1) * D],
                                    in_=w_in[c * 128:(c + 1) * 128, :])
            asr = cst.tile([1, D], f32)
            nc.scalar.dma_start(out=asr[:], in_=a_self[:])
            anr = cst.tile([1, D], f32)
            nc.scalar.dma_start(out=anr[:], in_=a_neigh[:])
            ones1 = cst.tile([1, 128], f32)
            nc.vector.memset(ones1[:], 1.0)

            # broadcast a_neigh / a_self across 128 partitions via PE
            pan = ps.tile([128, 2 * D], f32, tag="pro")
            nc.tensor.matmul(pan[:, 0:D], ones1[:], anr[:], start=True, stop=True)
            nc.tensor.matmul(pan[:, D:2 * D], ones1[:], asr[:], start=True, stop=True)
            anb = cst.tile([128, 2 * D], f32)
            nc.vector.tensor_copy(anb[:], pan[:])

            # W in bf16 hi + lo correction: z = f_bf16 @ wh + f_bf16 @ wl
            wh = cst.tile([128, 4 * D], bf16)
            nc.vector.tensor_copy(wh[:], w_sb[:])
            wl = cst.tile([128, 4 * D], bf16)
            nc.vector.tensor_tensor(wl[:], w_sb[:], wh[:], Alu.subtract)

            zsc = cst.tile([128, IB, D], f32)
            for ib in range(IB):
                psz = ps.tile([128, D], f32, tag="pro")
                first = True
                for wa in (wh, wl):
                    for c in range(4):
                        nc.tensor.matmul(
                            psz[:],
                            ft[:, c * R + ib * 128: c * R + (ib + 1) * 128],
                            wa[:, c * D:(c + 1) * D],
                            start=first, stop=(wa is wl and c == 3),
                        )
                        first = False
                nc.vector.tensor_copy(zsc[:, ib], psz[:])

            # t = z @ a_neigh, s = z @ a_self  (free-axis reduces)
            tscr = cst.tile([128, IB, D], f32)
            for ib in range(IB):
                nc.vector.tensor_tensor(tscr[:, ib], zsc[:, ib], anb[:, 0:D], Alu.mult)
            t_sb = cst.tile([128, IB], f32)
            nc.vector.tensor_reduce(t_sb[:], tscr[:], mybir.AxisListType.X, Alu.add)
            for ib in range(IB):
                nc.vector.tensor_tensor(tscr[:, ib], zsc[:, ib], anb[:, D:2 * D], Alu.mult)
            s_sb = cst.tile([128, IB], f32)
            nc.vector.tensor_reduce(s_sb[:], tscr[:], mybir.AxisListType.X, Alu.add)

            # s row: transpose [128, IB] -> [IB, 128] -> flat [1, R]
            ident = cst.tile([128, 128], f32)
            make_identity(nc, ident[:])
            pst = ps.tile([IB, 128], f32, tag="pro")
            nc.tensor.transpose(pst[:], s_sb[:], ident[:])
            st_sb = cst.tile([IB, 128], f32)
            nc.vector.tensor_copy(st_sb[:], pst[:])
            nc.sync.dma_start(
                out=s_out[:].rearrange("o (p c) -> (o p) c", p=IB), in_=st_sb[:])

            # et2 = exp(.2 t) f32; eq = exp(.8 t) f16 (shipped)
            et2 = cst.tile([128, IB], f32)
            nc.scalar.activation(et2[:], t_sb[:], Act.Exp, scale=ALPHA)
            eq_sb = cst.tile([128, IB], f32)
            nc.scalar.activation(eq_sb[:], t_sb[:], Act.Exp, scale=1.0 - ALPHA)
            nc.sync.dma_start(out=eq_out[:], in_=eq_sb[:])

            # z' = [z * et2 | et2 | pad] in f16
            za_sb = cst.tile([128, IB, ZW], f16)
            nc.vector.memset(za_sb[:], 0.0)
            for ib in range(IB):
                nc.vector.tensor_scalar_mul(
                    za_sb[:, ib, 0:D], zsc[:, ib], et2[:, ib:ib + 1])
            nc.vector.tensor_copy(za_sb[:, :, D], et2[:])
            nc.sync.dma_start(
                out=za_out[:].rearrange("(c p) d -> p c d", p=128), in_=za_sb[:])

    nc.compile()
    return nc


def _build_launch_b():
    import concourse.bacc as bacc
    import concourse.tile as tile
    from concourse import mybir

    f32 = mybir.dt.float32
    f16 = mybir.dt.float16
    Alu = mybir.AluOpType
    Act = mybir.ActivationFunctionType

    nc = bacc.Bacc("TRN2", target_bir_lowering=False, debug=False, num_devices=NCORES)

    a_t = nc.dram_tensor("a_t", [128, JC * R], f16, kind="ExternalInput")
    zaf = nc.dram_tensor("zaf", [128, JC * ZW], f16, kind="ExternalInput")
    s_in = nc.dram_tensor("s", [1, R], f32, kind="ExternalInput")
    eq_in = nc.dram_tensor("eq", [128, JC], f32, kind="ExternalInput")
    h_out = nc.dram_tensor("h", [R, D], f32, kind="ExternalOutput")

    schemes = [SCHEME16[jc % 16] for jc in range(JC)]

    with tile.TileContext(nc) as tc:
        with (
            tc.tile_pool(name="const", bufs=1) as cst,
            tc.tile_pool(name="ps_main", bufs=2, space="PSUM") as ps_main,
        ):
            # small inputs first: eq and s unblock the score chain early
            eq = cst.tile([128, JC], f32)
            nc.scalar.dma_start(out=eq[:], in_=eq_in[:])
            s_row = cst.tile([1, R], f32)
            nc.scalar.dma_start(out=s_row[:], in_=s_in[:])
            zf = cst.tile([128, JC, ZW], f16)        # z', j-chunked
            nc.sync.dma_start(
                out=zf[:], in_=zaf[:].rearrange("p (c d) -> p c d", d=ZW))
            ones1 = cst.tile([1, 128], f32)
            nc.vector.memset(ones1[:], 1.0)
            negone = cst.tile([128, 1], f32)
            nc.vector.memset(negone[:], -1.0)

            # s broadcast across partitions (PE), then p3 = exp(.8 s) f16
            psb = ps_main.tile([128, R], f32, tag="hp", name="psb")
            for hh in range(2):
                nc.tensor.matmul(
                    psb[:, hh * 512:(hh + 1) * 512],
                    ones1[:],
                    s_row[0:1, hh * 512:(hh + 1) * 512],
                    start=True, stop=True,
                )
            p3 = cst.tile([128, R], f16)
            nc.scalar.activation(p3[:], psb[:], Act.Exp, scale=1.0 - ALPHA)

            # two H' accumulators: even/odd chunks accumulate separately
            hps = [ps_main.tile([DP, R], f32, tag="hp", name=f"hp{g}")
                   for g in range(2)]
            # first/last chunk per parity group (for start/stop flags)
            first_par = {0: 0, 1: 1}
            last_par = {0: JC - 2, 1: JC - 1}

            # ---- main loop over j-chunks, A arrives 4 chunks per DMA ----
            with (
                tc.tile_pool(name="a_pool", bufs=3) as a_pool,
                tc.tile_pool(name="work", bufs=4) as work,
            ):
                dma_engines = [nc.sync, nc.scalar]
                a_tiles = {}
                for jc in range(JC):
                    if jc % CPD == 0:
                        blk = jc // CPD
                        atile = a_pool.tile([128, CPD * R], f16, tag="at")
                        dma_engines[blk % 2].dma_start(
                            out=atile[:],
                            in_=a_t[:, blk * CPD * R:(blk + 1) * CPD * R])
                        a_tiles[blk] = atile
                    at = a_tiles[jc // CPD][:, (jc % CPD) * R:(jc % CPD + 1) * R]

                    sch = schemes[jc]
                    par = jc % 2
                    hp = hps[par]
                    start = jc == first_par[par]
                    stop = jc == last_par[par]
                    zst = zf[:, jc, 0:DP]

                    if sch in ('X', 'W'):
                        # ea = max(p3*eq_j, 1) * A    -> 1 moving stream
                        m = work.tile([128, R], f16, tag="m")
                        nc.vector.tensor_scalar(
                            m[:], p3[:], eq[:, jc:jc + 1], 1.0,
                            Alu.mult, Alu.max)
                        ea = work.tile([128, R], f16, tag="ea")
                        eng = nc.gpsimd if sch == 'W' else nc.vector
                        eng.tensor_tensor(ea[:], m[:], at, Alu.mult)
                        for hh in range(2):
                            nc.tensor.matmul(
                                hp[:, hh * 512:(hh + 1) * 512],
                                zst, ea[:, hh * 512:(hh + 1) * 512],
                                start=start, stop=stop,
                            )
                    else:
                        # ea = A + relu(p3*eq_j - 1)*A -> 2 moving streams,
                        # same stationary + psum accumulator
                        for hh in range(2):
                            nc.tensor.matmul(
                                hp[:, hh * 512:(hh + 1) * 512],
                                zst, at[:, hh * 512:(hh + 1) * 512],
                                start=start, stop=False,
                            )
                        r = work.tile([128, R], f16, tag="m")
                        nc.scalar.activation(
                            r[:], p3[:], Act.Relu,
                            bias=negone[:, 0:1], scale=eq[:, jc:jc + 1])
                        g = work.tile([128, R], f16, tag="ea")
                        eng = nc.gpsimd if sch == 'Z' else nc.vector
                        eng.tensor_tensor(g[:], r[:], at, Alu.mult)
                        for hh in range(2):
                            nc.tensor.matmul(
                                hp[:, hh * 512:(hh + 1) * 512],
                                zst, g[:, hh * 512:(hh + 1) * 512],
                                start=False, stop=stop,
                            )

            # ---- epilogue: transpose H', normalize, store ----
            with (
                tc.tile_pool(name="epi", bufs=2) as epi,
            ):
                from concourse.masks import make_identity
                h_sb = cst.tile([DP, R], f32)
                nc.vector.tensor_copy(h_sb[:], hps[0][:])
                nc.vector.tensor_tensor(h_sb[:], h_sb[:], hps[1][:], Alu.add)
                ident = cst.tile([DP, DP], f32)
                make_identity(nc, ident[:])
                for b in range(R // 128):
                    trp = ps_main.tile([128, DP], f32, tag="hp")
                    nc.tensor.transpose(
                        trp[:], h_sb[:, b * 128:(b + 1) * 128], ident[:])
                    rec = epi.tile([128, 1], f32, tag="rec")
                    nc.vector.reciprocal(rec[:], trp[:, D:DP])
                    hb = epi.tile([128, D], f32, tag="hb")
                    nc.vector.tensor_scalar_mul(hb[:], trp[:, 0:D], rec[:, 0:1])
                    nc.sync.dma_start(
                        out=h_out[b * 128:(b + 1) * 128, :], in_=hb[:])

    nc.compile()
    return nc


def _get_programs():
    if "a" not in _CACHE:
        _CACHE["a"] = _build_launch_a()
        _CACHE["b"] = _build_launch_b()
    return _CACHE["a"], _CACHE["b"]


def _mask_to_f16(block):
    """0/1 int mask -> float16 exactly, fast (bit pattern 0x3C00 = 1.0)."""
    bits = (block != 0).astype(np.uint16) * np.uint16(0x3C00)
    return bits.view(np.float16)


def prepare_inputs_a(features, W, a_self, a_neigh):
    features = np.asarray(features, dtype=np.float32)
    feat_bf = _f32_to_bf16(features)
    W = np.ascontiguousarray(np.asarray(W, dtype=np.float32))
    a_self_r = np.ascontiguousarray(np.asarray(a_self, dtype=np.float32).reshape(1, D))
    a_neigh_r = np.ascontiguousarray(np.asarray(a_neigh, dtype=np.float32).reshape(1, D))
    in_a = []
    for k in range(NCORES):
        rows = slice(k * R, (k + 1) * R)
        in_a.append({
            "feat_t": np.ascontiguousarray(feat_bf[rows, :].T),
            "w": W,
            "a_self": a_self_r,
            "a_neigh": a_neigh_r,
        })
    return in_a


def _f32_to_bf16(x):
    import ml_dtypes
    return x.astype(ml_dtypes.bfloat16)


def prepare_inputs_b(A, res_a):
    za_rows = np.concatenate([res_a[k]["za"] for k in range(NCORES)], axis=0)
    # B-layout: zaf[p, c*ZW+d] = z'[c*128+p, d]
    zaf = np.ascontiguousarray(
        za_rows.reshape(JC, 128, ZW).transpose(1, 0, 2).reshape(128, JC * ZW))
    eq_full = np.ascontiguousarray(
        np.concatenate([res_a[k]["eq"] for k in range(NCORES)], axis=1))
    in_b = []
    for k in range(NCORES):
        rows = slice(k * R, (k + 1) * R)
        blk = _mask_to_f16(np.asarray(A[rows, :]))      # [R, N] 0/1 f16
        # at[p, jc*R + i] = A[k*R + i, jc*128 + p]
        at = np.ascontiguousarray(
            blk.reshape(R, JC, 128).transpose(2, 1, 0).reshape(128, JC * R))
        in_b.append({
            "a_t": at,
            "zaf": zaf,
            "s": res_a[k]["s"],
            "eq": eq_full,
        })
    return in_b


def kernel(features, A, W, a_self, a_neigh):
    from concourse.bass_utils import run_bass_kernel_spmd

    nca, ncb = _get_programs()
    in_a = prepare_inputs_a(features, W, a_self, a_neigh)
    res_a = run_bass_kernel_spmd(nca, in_a, list(range(NCORES))).results
    in_b = prepare_inputs_b(A, res_a)
    res_b = run_bass_kernel_spmd(ncb, in_b, list(range(NCORES))).results
    h = np.concatenate([res_b[k]["h"] for k in range(NCORES)], axis=0)
    return h.astype(np.float32)


# revision 8
# speedup vs baseline: 1.1047x; 1.1047x over previous
"""GAT layer (nn_GATLayer) on 8 Trainium2 NeuronCores via Bass/Tile.

Reference computation (N=8192, F=512, D=64):
    z = features @ W                      # [N, D]
    s = z @ a_self; t = z @ a_neigh       # [N, 1]
    e[i,j] = leakyrelu(s[i] + t[j], 0.2)
    attention = softmax(e + mask(A), axis=1)   # mask: -1e12 where A<=0
    h = attention @ z                     # [N, D]

Row-sharded across 8 cores (1024 attention rows each), two launches with a
host-side gather between them (replaces an on-device AllGather whose
rendezvous barrier alone costs ~47 us).

Key algebra: with e = s_i + t_j, q_j = exp(.8 t_j), p_i = exp(.8 s_i),
    exp(leakyrelu(e)) = exp(.2 t_j) * exp(.2 s_i) * max(p_i q_j, 1).
The exp(.2 s_i) column factor cancels between softmax numerator and
denominator and is dropped; exp(.2 t_j) is folded into the stationary
z' = [z | 1] * exp(.2 t_j) in launch A.  Per-tile mask weight is then
    ea = A * max(pq, 1)
and the crucial split (scheme Y/Z below)
    ea = A + relu(pq - 1) * A
lets the raw A tile stream straight from DMA into the PE (zero elementwise
cost) with the relu term as a second moving operand into the SAME psum
accumulator + stationary.  relu(pq - 1) is ONE scalar-engine activation:
Relu(p3 * eq_j - 1) via the per-partition `scale` operand.

Launch A (small): each core computes z (bf16 one-pass + W-lo correction),
t, s for its own 1024 rows; ships z' pre-scaled/cast to f16, eq = exp(.8t)
f16, s row f32.

Launch B (main): per j-chunk (128 j's x 1024 i's), one of three schemes
balances the elementwise mask work across engines under the A-DMA roofline:
  X: m  = max(p3*eq_j, 1)        (DVE tensor_scalar, 1 op)
     ea = m * A                  (DVE tensor_tensor) -> 1 moving stream
  Y: r  = Relu(p3*eq_j - 1)      (ACT activation, 1 op)
     g  = r * A                  (DVE tensor_tensor) -> 2 moving streams
  W/Z: same as X/Y but the tensor_tensor runs on GpSimd.
A-tiles arrive as 1 MB DMAs (4 chunks each) alternating sync/scalar HWDGE
queues; A ships as float16 0/1 (exact), host-prepacked to the on-chip
layout [128, JC*1024].
"""

import sys

sys.path.insert(0, "/opt/trn_rl_repo")

import numpy as np

N, F, D = 8192, 512, 64
NCORES = 8
R = N // NCORES          # rows per core (1024)
JC = N // 128            # j-chunks (64)
DP = D + 1               # z' | et2  (65)
ZW = 80                  # padded z' width (80 f16 = 160B rows)
ALPHA = 0.2
CPD = 4                  # chunks per A-DMA (4 * 256KB = 1MB)

# per-16-chunk scheme pattern (all single PE stream):
#   'V' = DVE ts + DVE tt (V chunks come in adjacent pairs -> batched tt)
#   'A' = ACT relu+exp pair      'G' = DVE ts + GpSimd tt
SCHEME16 = ['A', 'G', 'V', 'V', 'A', 'V', 'V', 'A',
            'V', 'V', 'A', 'G', 'G', 'A', 'V', 'V']

_CACHE = {}


def _build_launch_a():
    """Per-core z' = [z|1]*exp(.2 t) f16, eq = exp(.8 t) f16, s row f32."""
    import concourse.bacc as bacc
    import concourse.tile as tile
    from concourse import mybir
    from concourse.masks import make_identity

    f32 = mybir.dt.float32
    f16 = mybir.dt.float16
    bf16 = mybir.dt.bfloat16
    Alu = mybir.AluOpType
    Act = mybir.ActivationFunctionType

    nc = bacc.Bacc("TRN2", target_bir_lowering=False, debug=False, num_devices=NCORES)

    feat_t = nc.dram_tensor("feat_t", [F, R], bf16, kind="ExternalInput")
    w_in = nc.dram_tensor("w", [F, D], f32, kind="ExternalInput")
    a_self = nc.dram_tensor("a_self", [1, D], f32, kind="ExternalInput")
    a_neigh = nc.dram_tensor("a_neigh", [1, D], f32, kind="ExternalInput")
    za_out = nc.dram_tensor("za", [R, ZW], f16, kind="ExternalOutput")
    s_out = nc.dram_tensor("s", [1, R], f32, kind="ExternalOutput")
    eq_out = nc.dram_tensor("eq", [128, R // 128], f32, kind="ExternalOutput")
    t_out = nc.dram_tensor("t", [128, R // 128], f32, kind="ExternalOutput")

    IB = R // 128  # 8 row-blocks per core

    with tile.TileContext(nc) as tc:
        with (
            tc.tile_pool(name="sb", bufs=1) as cst,
            tc.tile_pool(name="ps", bufs=2, space="PSUM") as ps,
        ):
            ft = cst.tile([128, 4 * R], bf16)
            for c in range(4):
                nc.sync.dma_start(out=ft[:, c * R:(c + 1) * R],
                                  in_=feat_t[c * 128:(c + 1) * 128, :])
            w_sb = cst.tile([128, 4 * D], f32)
            for c in range(4):
                nc.scalar.dma_start(out=w_sb[:, c * D:(c + 1) * D],
                                    in_=w_in[c * 128:(c + 1) * 128, :])
            asr = cst.tile([1, D], f32)
            nc.scalar.dma_start(out=asr[:], in_=a_self[:])
            anr = cst.tile([1, D], f32)
            nc.scalar.dma_start(out=anr[:], in_=a_neigh[:])
            ones1 = cst.tile([1, 128], f32)
            nc.vector.memset(ones1[:], 1.0)

            # broadcast a_neigh / a_self across 128 partitions via PE
            pan = ps.tile([128, 2 * D], f32, tag="pro")
            nc.tensor.matmul(pan[:, 0:D], ones1[:], anr[:], start=True, stop=True)
            nc.tensor.matmul(pan[:, D:2 * D], ones1[:], asr[:], start=True, stop=True)
            anb = cst.tile([128, 2 * D], f32)
            nc.vector.tensor_copy(anb[:], pan[:])

            # W in bf16 hi + lo correction: z = f_bf16 @ wh + f_bf16 @ wl
            wh = cst.tile([128, 4 * D], bf16)
            nc.vector.tensor_copy(wh[:], w_sb[:])
            wl = cst.tile([128, 4 * D], bf16)
            nc.vector.tensor_tensor(wl[:], w_sb[:], wh[:], Alu.subtract)

            zsc = cst.tile([128, IB, D], f32)
            for ib in range(IB):
                psz = ps.tile([128, D], f32, tag="pro")
                first = True
                for wa in (wh, wl):
                    for c in range(4):
                        nc.tensor.matmul(
                            psz[:],
                            ft[:, c * R + ib * 128: c * R + (ib + 1) * 128],
                            wa[:, c * D:(c + 1) * D],
                            start=first, stop=(wa is wl and c == 3),
                        )
                        first = False
                nc.vector.tensor_copy(zsc[:, ib], psz[:])

            # t = z @ a_neigh, s = z @ a_self  (free-axis reduces)
            tscr = cst.tile([128, IB, D], f32)
            for ib in range(IB):
                nc.vector.tensor_tensor(tscr[:, ib], zsc[:, ib], anb[:, 0:D], Alu.mult)
            t_sb = cst.tile([128, IB], f32)
            nc.vector.tensor_reduce(t_sb[:], tscr[:], mybir.AxisListType.X, Alu.add)
            for ib in range(IB):
                nc.vector.tensor_tensor(tscr[:, ib], zsc[:, ib], anb[:, D:2 * D], Alu.mult)
            s_sb = cst.tile([128, IB], f32)
            nc.vector.tensor_reduce(s_sb[:], tscr[:], mybir.AxisListType.X, Alu.add)

            # s row: transpose [128, IB] -> [IB, 128] -> flat [1, R]
            ident = cst.tile([128, 128], f32)
            make_identity(nc, ident[:])
            pst = ps.tile([IB, 128], f32, tag="pro")
            nc.tensor.transpose(pst[:], s_sb[:], ident[:])
            st_sb = cst.tile([IB, 128], f32)
            nc.vector.tensor_copy(st_sb[:], pst[:])
            nc.sync.dma_start(
                out=s_out[:].rearrange("o (p c) -> (o p) c", p=IB), in_=st_sb[:])

            # et2 = exp(.2 t) f32; eq = exp(.8 t) f16 (shipped)
            et2 = cst.tile([128, IB], f32)
            nc.scalar.activation(et2[:], t_sb[:], Act.Exp, scale=ALPHA)
            eq_sb = cst.tile([128, IB], f32)
            nc.scalar.activation(eq_sb[:], t_sb[:], Act.Exp, scale=1.0 - ALPHA)
            nc.sync.dma_start(out=eq_out[:], in_=eq_sb[:])
            nc.sync.dma_start(out=t_out[:], in_=t_sb[:])

            # z' = [z * et2 | et2 | pad] in f16
            za_sb = cst.tile([128, IB, ZW], f16)
            nc.vector.memset(za_sb[:], 0.0)
            for ib in range(IB):
                nc.vector.tensor_scalar_mul(
                    za_sb[:, ib, 0:D], zsc[:, ib], et2[:, ib:ib + 1])
            nc.vector.tensor_copy(za_sb[:, :, D], et2[:])
            nc.sync.dma_start(
                out=za_out[:].rearrange("(c p) d -> p c d", p=128), in_=za_sb[:])

    nc.compile()
    return nc


def _build_launch_b():
    import concourse.bacc as bacc
    import concourse.tile as tile
    from concourse import mybir

    f32 = mybir.dt.float32
    f16 = mybir.dt.float16
    Alu = mybir.AluOpType
    Act = mybir.ActivationFunctionType

    nc = bacc.Bacc("TRN2", target_bir_lowering=False, debug=False, num_devices=NCORES)

    a_t = nc.dram_tensor("a_t", [128, JC * R], f16, kind="ExternalInput")
    zaf = nc.dram_tensor("zaf", [128, JC * ZW], f16, kind="ExternalInput")
    s_in = nc.dram_tensor("s", [1, R], f32, kind="ExternalInput")
    eq_in = nc.dram_tensor("eq", [128, JC], f32, kind="ExternalInput")
    t_in = nc.dram_tensor("t", [128, JC], f32, kind="ExternalInput")
    h_out = nc.dram_tensor("h", [R, D], f32, kind="ExternalOutput")

    schemes = [SCHEME16[jc % 16] for jc in range(JC)]

    with tile.TileContext(nc) as tc:
        with (
            tc.tile_pool(name="const", bufs=1) as cst,
            tc.tile_pool(name="ps_main", bufs=2, space="PSUM") as ps_main,
        ):
            # small inputs first: eq/t and s unblock the score chain early
            eq = cst.tile([128, JC], f32)
            nc.scalar.dma_start(out=eq[:], in_=eq_in[:])
            tt_sb = cst.tile([128, JC], f32)
            nc.scalar.dma_start(out=tt_sb[:], in_=t_in[:])
            s_row = cst.tile([1, R], f32)
            nc.scalar.dma_start(out=s_row[:], in_=s_in[:])
            zf = cst.tile([128, JC, ZW], f16)        # z', j-chunked
            nc.sync.dma_start(
                out=zf[:], in_=zaf[:].rearrange("p (c d) -> p c d", d=ZW))
            ones1 = cst.tile([1, 128], f32)
            nc.vector.memset(ones1[:], 1.0)

            # t8 = .8 t (bias for the ACT relu chunks)
            t8 = cst.tile([128, JC], f32)
            nc.scalar.activation(t8[:], tt_sb[:], Act.Identity,
                                 scale=1.0 - ALPHA)

            # s broadcast across partitions (PE); s_bcast f32 + p3 f16
            psb = ps_main.tile([128, R], f32, tag="hp", name="psb")
            for hh in range(2):
                nc.tensor.matmul(
                    psb[:, hh * 512:(hh + 1) * 512],
                    ones1[:],
                    s_row[0:1, hh * 512:(hh + 1) * 512],
                    start=True, stop=True,
                )
            s_bcast = cst.tile([128, R], f32)
            nc.vector.tensor_copy(s_bcast[:], psb[:])
            p3 = cst.tile([128, R], f16)
            nc.scalar.activation(p3[:], s_bcast[:], Act.Exp, scale=1.0 - ALPHA)

            # two H' accumulators: even/odd chunks accumulate separately
            hps = [ps_main.tile([DP, R], f32, tag="hp", name=f"hp{g}")
                   for g in range(2)]

            # ---- main loop over j-chunks, A arrives 4 chunks per DMA ----
            with (
                tc.tile_pool(name="a_pool", bufs=3) as a_pool,
                tc.tile_pool(name="work", bufs=4) as work,
            ):
                dma_engines = [nc.sync, nc.scalar]
                a_tiles = {}
                pend_pair = None
                for jc in range(JC):
                    if jc % CPD == 0:
                        blk = jc // CPD
                        atile = a_pool.tile([128, CPD * R], f16, tag="at")
                        dma_engines[blk % 2].dma_start(
                            out=atile[:],
                            in_=a_t[:, blk * CPD * R:(blk + 1) * CPD * R])
                        a_tiles[blk] = atile
                    at = a_tiles[jc // CPD][:, (jc % CPD) * R:(jc % CPD + 1) * R]

                    sch = schemes[jc]
                    par = jc % 2
                    hp = hps[par]
                    start = jc in (0, 1)
                    stop = jc in (JC - 2, JC - 1)
                    zst = zf[:, jc, 0:DP]

                    if sch == 'V' and pend_pair is None and \
                            jc + 1 < JC and schemes[jc + 1] == 'V' and \
                            (jc % CPD) + 1 < CPD:
                        # first of an adjacent V pair: batch the tt over 2 chunks
                        m2 = work.tile([128, 2 * R], f16, tag="m2")
                        nc.vector.tensor_scalar(
                            m2[:, 0:R], p3[:], eq[:, jc:jc + 1], 1.0,
                            Alu.mult, Alu.max)
                        nc.vector.tensor_scalar(
                            m2[:, R:2 * R], p3[:], eq[:, jc + 1:jc + 2], 1.0,
                            Alu.mult, Alu.max)
                        ea2 = work.tile([128, 2 * R], f16, tag="ea2")
                        at2 = a_tiles[jc // CPD][
                            :, (jc % CPD) * R:(jc % CPD + 2) * R]
                        nc.vector.tensor_tensor(ea2[:], m2[:], at2, Alu.mult)
                        pend_pair = ea2
                        ea_sl = ea2[:, 0:R]
                    elif pend_pair is not None:
                        ea_sl = pend_pair[:, R:2 * R]
                        pend_pair = None
                    else:
                        if sch in ('V', 'G'):
                            m = work.tile([128, R], f16, tag="m")
                            nc.vector.tensor_scalar(
                                m[:], p3[:], eq[:, jc:jc + 1], 1.0,
                                Alu.mult, Alu.max)
                        else:  # ACT pair: u = relu(.8(s+t)); m = exp(u)
                            u = work.tile([128, R], f32, tag="u")
                            nc.scalar.activation(
                                u[:], s_bcast[:], Act.Relu,
                                bias=t8[:, jc:jc + 1], scale=1.0 - ALPHA)
                            m = work.tile([128, R], f16, tag="m")
                            nc.scalar.activation(m[:], u[:], Act.Exp)
                        ea = work.tile([128, R], f16, tag="ea")
                        eng = nc.gpsimd if sch == 'G' else nc.vector
                        eng.tensor_tensor(ea[:], m[:], at, Alu.mult)
                        ea_sl = ea[:]
                    for hh in range(2):
                        nc.tensor.matmul(
                            hp[:, hh * 512:(hh + 1) * 512],
                            zst, ea_sl[:, hh * 512:(hh + 1) * 512],
                            start=start, stop=stop,
                        )

            # ---- epilogue: transpose H', normalize, store ----
            with (
                tc.tile_pool(name="epi", bufs=2) as epi,
            ):
                from concourse.masks import make_identity
                h_sb = cst.tile([DP, R], f32)
                nc.vector.tensor_copy(h_sb[:], hps[0][:])
                nc.vector.tensor_tensor(h_sb[:], h_sb[:], hps[1][:], Alu.add)
                ident = cst.tile([DP, DP], f32)
                make_identity(nc, ident[:])
                for b in range(R // 128):
                    trp = ps_main.tile([128, DP], f32, tag="hp")
                    nc.tensor.transpose(
                        trp[:], h_sb[:, b * 128:(b + 1) * 128], ident[:])
                    rec = epi.tile([128, 1], f32, tag="rec")
                    nc.vector.reciprocal(rec[:], trp[:, D:DP])
                    hb = epi.tile([128, D], f32, tag="hb")
                    nc.vector.tensor_scalar_mul(hb[:], trp[:, 0:D], rec[:, 0:1])
                    nc.sync.dma_start(
                        out=h_out[b * 128:(b + 1) * 128, :], in_=hb[:])

    nc.compile()
    return nc


def _get_programs():
    if "a" not in _CACHE:
        _CACHE["a"] = _build_launch_a()
        _CACHE["b"] = _build_launch_b()
    return _CACHE["a"], _CACHE["b"]


def _mask_to_f16(block):
    """0/1 int mask -> float16 exactly, fast (bit pattern 0x3C00 = 1.0)."""
    bits = (block != 0).astype(np.uint16) * np.uint16(0x3C00)
    return bits.view(np.float16)


def prepare_inputs_a(features, W, a_self, a_neigh):
    features = np.asarray(features, dtype=np.float32)
    feat_bf = _f32_to_bf16(features)
    W = np.ascontiguousarray(np.asarray(W, dtype=np.float32))
    a_self_r = np.ascontiguousarray(np.asarray(a_self, dtype=np.float32).reshape(1, D))
    a_neigh_r = np.ascontiguousarray(np.asarray(a_neigh, dtype=np.float32).reshape(1, D))
    in_a = []
    for k in range(NCORES):
        rows = slice(k * R, (k + 1) * R)
        in_a.append({
            "feat_t": np.ascontiguousarray(feat_bf[rows, :].T),
            "w": W,
            "a_self": a_self_r,
            "a_neigh": a_neigh_r,
        })
    return in_a


def _f32_to_bf16(x):
    import ml_dtypes
    return x.astype(ml_dtypes.bfloat16)


def prepare_inputs_b(A, res_a):
    za_rows = np.concatenate([res_a[k]["za"] for k in range(NCORES)], axis=0)
    # B-layout: zaf[p, c*ZW+d] = z'[c*128+p, d]
    zaf = np.ascontiguousarray(
        za_rows.reshape(JC, 128, ZW).transpose(1, 0, 2).reshape(128, JC * ZW))
    eq_full = np.ascontiguousarray(
        np.concatenate([res_a[k]["eq"] for k in range(NCORES)], axis=1))
    t_full = np.ascontiguousarray(
        np.concatenate([res_a[k]["t"] for k in range(NCORES)], axis=1))
    in_b = []
    for k in range(NCORES):
        rows = slice(k * R, (k + 1) * R)
        blk = _mask_to_f16(np.asarray(A[rows, :]))      # [R, N] 0/1 f16
        # at[p, jc*R + i] = A[k*R + i, jc*128 + p]
        at = np.ascontiguousarray(
            blk.reshape(R, JC, 128).transpose(2, 1, 0).reshape(128, JC * R))
        in_b.append({
            "a_t": at,
            "zaf": zaf,
            "s": res_a[k]["s"],
            "eq": eq_full,
            "t": t_full,
        })
    return in_b


def kernel(features, A, W, a_self, a_neigh):
    from concourse.bass_utils import run_bass_kernel_spmd

    nca, ncb = _get_programs()
    in_a = prepare_inputs_a(features, W, a_self, a_neigh)
    res_a = run_bass_kernel_spmd(nca, in_a, list(range(NCORES))).results
    in_b = prepare_inputs_b(A, res_a)
    res_b = run_bass_kernel_spmd(ncb, in_b, list(range(NCORES))).results
    h = np.concatenate([res_b[k]["h"] for k in range(NCORES)], axis=0)
    return h.astype(np.float32)
